# revision 22
# baseline (speedup 1.0000x reference)
"""Trainium2 Bass kernel for windowed (block-diagonal) multi-head attention.

Problem nn_Attention_17059610099953:
  x: (8, 1936, 384) tokens of a (B=2, t=4, H=44, W=44) volume; 10x10 spatial
  windows (padded to 50x50 -> 5x5 grid), each window = t*10*10 = 400 tokens of
  12-head attention (head_dim 32), followed by an output projection.

Sharding: 50 windows = 32 full (400 real tokens) + 16 edge (160) + 2 corner
(64). Each of the 8 NeuronCores processes 4 full windows + 2 edge windows
(compacted to 160 tokens) + 1 corner slot (64 tokens; zero for cores 2-7).
The softmax denominator is corrected by +(400 - slot_n) to match the
reference's 400-slot windows, whose zero padding tokens each contribute
exp(0)=1.

v3 pipeline (baseline v2 was 234us):
  - All matmul operands bf16; PSUM accumulation fp32.
  - QK^T runs pair-wise with PE row-strip tiling: heads (2p, 2p+1) target
    disjoint 32-row groups (tile_position), so their K=32 matmuls execute
    concurrently in the systolic array (~2x QK wall time).
  - exp is split between the Scalar engine (ACT exp LUT) and the Vector
    engine (Schraudolph int16 bitcast) per (head, j-group) half; the split
    is chosen to balance ACT vs DVE load within the 2e-2 error budget.
  - PV is column-tiled: heads 2p/2p+1 run in column strips (0,0)/(0,64);
    the 33rd stationary column is the all-ones softmax-denominator row.
  - PV output copies grab rows 0:97 in one instruction per pair (DVE cost
    scales with free size, not partitions), halving copy count.
  - All 12 per-head rescale muls run on GpSimd; normalization gathers and
    broadcasts ride the GpSimd DMA queue.
  - The PE instruction stream interleaves next-window QKV projections and
    prev-window tails between QK/PV groups to avoid >3.4us idle gaps that
    would re-throttle the HAM clock gate to 1.2 GHz.
"""
import os
import sys

for _p in ("/opt/trn_rl_repo",):
    if os.path.isdir(_p) and _p not in sys.path:
        sys.path.append(_p)

import numpy as np
import ml_dtypes

import concourse.bass as bass
import concourse.bacc as bacc
import concourse.mybir as mybir
import concourse.tile as tile

F32 = mybir.dt.float32
F32R = mybir.dt.float32r
BF16 = mybir.dt.bfloat16
I16 = mybir.dt.int16
AF = mybir.ActivationFunctionType
ALU = mybir.AluOpType

C = 384
NH = 12
HD = 32
SCALE = HD ** -0.5
NF = 4      # full windows per core (n=400)
NS = 2      # edge windows per core (n=160)
NFull = 400
NSmall = 160
NCorner = 64
VW = 33     # V columns per head (32 dims + ones column for the denominator)

# Schraudolph exp constants, bf16 flavor:
# exp(x) ~= bitcast_bf16(int16(x*EXPA16 + EXPB16))
EXPA16 = 184.6649652337873     # 2^7 / ln 2
EXPB16 = 127.0 * 128.0 - 5.585


def ceil_div(a, b):
    return (a + b - 1) // b


# exp units whose HIGH pair (tile B) is computed on the Vector engine
# (Schraudolph) while the low pair stays on Scalar. Keys are (quad, j).
# Splitting a unit across engines frees both score tiles simultaneously,
# which keeps the 4-strip QK quads concurrent; more units on DVE unloads
# ACT but raises the output error.
DVE_FULL = frozenset({(0, 2), (1, 3), (2, 1)})
DVE_SMALL = frozenset({(1, 1), (2, 0)})
DVE_CORNER = frozenset()
MUL_DVE = frozenset({1, 3, 5, 7, 9, 11})


def build_kernel(dve_full=DVE_FULL, dve_small=DVE_SMALL,
                 dve_corner=DVE_CORNER, mul_dve=MUL_DVE, n_warm=0):
    nc = bacc.Bacc("TRN2", target_bir_lowering=False, debug=False, num_devices=8)

    xf = nc.declare_dram_parameter("xf", [NF, 128, 3, NFull], BF16, isOutput=False)
    xs = nc.declare_dram_parameter("xs", [NS, 128, 3, NSmall], BF16, isOutput=False)
    xc = nc.declare_dram_parameter("xc", [1, 128, 3, NCorner], BF16, isOutput=False)
    wq = nc.declare_dram_parameter("wq", [128, 3, C], BF16, isOutput=False)
    wk = nc.declare_dram_parameter("wk", [128, 3, C], BF16, isOutput=False)
    wv = nc.declare_dram_parameter("wv", [128, 3, C], BF16, isOutput=False)
    wp = nc.declare_dram_parameter("wp", [128, 3, C], BF16, isOutput=False)
    pb = nc.declare_dram_parameter("pb", [128, 3], F32, isOutput=False)
    zf = nc.declare_dram_parameter("zf", [NF, 128, 3, NFull], BF16, isOutput=True)
    zs = nc.declare_dram_parameter("zs", [NS, 128, 3, NSmall], BF16, isOutput=True)
    zc = nc.declare_dram_parameter("zc", [1, 128, 3, NCorner], BF16, isOutput=True)

    slots = [(s, NFull, xf, zf, s, dve_full) for s in range(NF)] + \
            [(NF + s, NSmall, xs, zs, s, dve_small) for s in range(NS)] + \
            [(NF + NS, NCorner, xc, zc, 0, dve_corner)]
    NW = len(slots)

    with tile.TileContext(nc) as tc:
        with tc.tile_pool(name="weights", bufs=1) as wpool, \
             tc.tile_pool(name="xio", bufs=5) as xpool, \
             tc.tile_pool(name="qk", bufs=3) as qkpool, \
             tc.tile_pool(name="vaug", bufs=5) as vpool, \
             tc.tile_pool(name="es", bufs=8) as espool, \
             tc.tile_pool(name="oun", bufs=4) as ounpool, \
             tc.tile_pool(name="oz", bufs=4) as ozpool, \
             tc.tile_pool(name="nrm", bufs=3) as nrmpool, \
             tc.tile_pool(name="nrmbig", bufs=4) as nbpool, \
             tc.tile_pool(name="dscratch", bufs=6, space="DRAM") as dpool, \
             tc.tile_pool(name="ps_s", bufs=2, space="PSUM") as ps_s, \
             tc.tile_pool(name="ps_pv", bufs=2, space="PSUM") as ps_pv, \
             tc.tile_pool(name="ps_mm", bufs=2, space="PSUM") as ps_mm:

            twq = wpool.tile([128, 3, C], BF16, tag="wq")
            twk = wpool.tile([128, 3, C], BF16, tag="wk")
            twv = wpool.tile([128, 3, C], BF16, tag="wv")
            twp = wpool.tile([128, 3, C], BF16, tag="wp")
            tpb = wpool.tile([128, 3], F32, tag="pb")

            class Window:
                def __init__(self, w):
                    self.w = w
                    (self.slot, self.n, self.xin, self.zout, self.si,
                     self.dve_set) = slots[w]
                    self.n_mt = ceil_div(self.n, 128)
                    self.m_sizes = [min(128, self.n - 128 * j)
                                    for j in range(self.n_mt)]
                    self.full = self.n == NFull
                    self.in_last = False
                    self.es = {}
                    self.pss = {}

                # ---- stage 1: x load + QKV projections (run during w-1) ----
                def load_x(self):
                    self.xt = xpool.tile([128, 3, NFull], BF16, tag="xt",
                                         name=f"xt{self.w}")
                    nc.sync.dma_start(out=self.xt[:, :, 0:self.n],
                                      in_=self.xin[self.si])

                def qkv_chunks(self):
                    n = self.n
                    out = []

                    def qk_proj(dst_key, i, self=self):
                        if dst_key not in ("qt", "kt"):
                            raise ValueError
                        if not hasattr(self, dst_key):
                            setattr(self, dst_key,
                                    qkpool.tile([128, 3, NFull], BF16,
                                                tag=dst_key,
                                                name=f"{dst_key}{self.w}"))
                        dst = getattr(self, dst_key)
                        w_t = twq if dst_key == "qt" else twk
                        pmm = ps_mm.tile([128, 512], F32, tag="mm")
                        for kk in range(3):
                            nc.tensor.matmul(pmm[:, 0:n],
                                             w_t[:, kk, 128 * i:128 * i + 128],
                                             self.xt[:, kk, 0:n],
                                             start=(kk == 0), stop=(kk == 2))
                        with tc.high_priority(offset=10**6):
                            nc.vector.tensor_copy(dst[:, i, 0:n],
                                                  pmm[:, 0:n])

                    def v_tile(j, self=self):
                        if not hasattr(self, "vg"):
                            self.vg = vpool.tile([128, 4, NH * VW], BF16,
                                                 tag="vg", name=f"vg{self.w}")
                            vs = self.vg.rearrange("p j (h c) -> p j h c", h=NH)
                            nc.vector.memset(
                                vs[:, 0:self.n_mt, :, 32:33], 1.0)
                        mj = self.m_sizes[j]
                        pmm = ps_mm.tile([128, 512], F32, tag="mm")
                        for kk in range(3):
                            nc.tensor.matmul(pmm[0:mj, 0:C],
                                             self.xt[:, kk, 128 * j:128 * j + mj],
                                             twv[:, kk, :],
                                             start=(kk == 0), stop=(kk == 2))
                        vslice = self.vg[0:mj, j, :].rearrange(
                            "p (h c) -> p h c", h=NH)
                        with tc.high_priority(offset=10**6):
                            nc.vector.tensor_copy(
                                vslice[:, :, 0:32],
                                pmm[0:mj, 0:C].rearrange("p (h c) -> p h c",
                                                         h=NH))

                    for dst in ("qt", "kt"):
                        for i in range(3):
                            out.append(lambda d=dst, i=i: qk_proj(d, i))
                    for j in range(self.n_mt):
                        out.append(lambda j=j: v_tile(j))
                    return out

                # ---- stage 2: quad QK^T (4 row strips) + exp ----
                # Heads 4Q..4Q+3 run concurrently in the four 32-row strips
                # of the PE array (same kt/qt chunk ti=Q), writing 4
                # distinct PSUM banks: head pair (4Q,4Q+1) -> tile A slots
                # 0/1, (4Q+2,4Q+3) -> tile B. This uses the full array for
                # the K=32 score matmuls (4x fewer array-cycles than
                # serial heads).
                def qk_quad(self, Q, j):
                    n, n_mt = self.n, self.n_mt
                    if j >= n_mt:
                        return
                    mj = self.m_sizes[j]
                    p_lo, p_hi = 2 * Q, 2 * Q + 1
                    for p in (p_lo, p_hi):
                        if p not in self.es:
                            self.es[p] = espool.tile(
                                [128, 2, 4, NFull], BF16, tag="es",
                                name=f"es{self.w}_{p}")
                    tA = ps_s.tile([128, 2, 512], F32, tag="s",
                                   name=f"pssA{self.w}_{Q}_{j}")
                    tB = ps_s.tile([128, 2, 512], F32, tag="s",
                                   name=f"pssB{self.w}_{Q}_{j}")
                    for hi in range(4):
                        h = 4 * Q + hi
                        to = 32 * hi
                        dst = (tA, tB)[hi // 2][0:mj, hi % 2, 0:n]
                        nc.tensor.matmul(
                            dst,
                            self.kt[to:to + 32, Q, 128 * j:128 * j + mj],
                            self.qt[to:to + 32, Q, 0:n],
                            start=True, stop=True, tile_position=(to, 0))
                    for hi_pair, (p, t) in enumerate(((p_lo, tA),
                                                      (p_hi, tB))):
                        dst = self.es[p][:, 0:2, j, 0:n]
                        src = t[:, 0:2, 0:n]
                        with tc.high_priority(offset=10**6):
                            if (Q, j) in self.dve_set:
                                nc.vector.tensor_scalar(
                                    dst.bitcast(I16), src,
                                    SCALE * EXPA16, EXPB16, ALU.mult, ALU.add)
                            else:
                                nc.scalar.activation(dst, src, AF.Exp,
                                                     scale=SCALE)

                # ---- stage 3: PV for a head pair, column-tiled ----
                def pv_pair(self, p):
                    n, n_mt = self.n, self.n_mt
                    h0, h1 = 2 * p, 2 * p + 1
                    if not hasattr(self, "oun"):
                        self.oun = ounpool.tile([128, 6, NFull], BF16,
                                                tag="oun", name=f"oun{self.w}")
                    esp = self.es.pop(p)
                    ppv = ps_pv.tile([128, 512], F32, tag="pv")
                    for j in range(n_mt):
                        mj = self.m_sizes[j]
                        nc.tensor.matmul(
                            ppv[0:33, 0:n],
                            self.vg[0:mj, j, VW * h0:VW * h0 + VW],
                            esp[0:mj, 0, j, 0:n],
                            start=(j == 0), stop=(j == n_mt - 1),
                            tile_position=(0, 0), skip_group_check=True)
                        nc.tensor.matmul(
                            ppv[64:97, 0:n],
                            self.vg[0:mj, j, VW * h1:VW * h1 + VW],
                            esp[0:mj, 1, j, 0:n],
                            start=(j == 0), stop=(j == n_mt - 1),
                            tile_position=(0, 64), skip_group_check=True)
                    # one copy for both heads: DVE cost scales with the free
                    # dim, so grabbing rows 0:97 (33:64 are dead) is as cheap
                    # as one head's 0:33
                    with tc.high_priority(offset=10**6):
                        if self.in_last:
                            nc.scalar.copy(self.oun[0:97, p, 0:n],
                                           ppv[0:97, 0:n])
                        else:
                            nc.vector.tensor_copy(self.oun[0:97, p, 0:n],
                                                  ppv[0:97, 0:n])

                # ---- stage 4: normalize + project + store (run during w+1) --
                def t_dal(self):
                    n = self.n
                    self.dal = nrmpool.tile([12, NFull], BF16, tag="dal",
                                            name=f"dal{self.w}")
                    nc.gpsimd.dma_start(out=self.dal[0:6, 0:n],
                                        in_=self.oun[32:33, :, 0:n])
                    nc.gpsimd.dma_start(out=self.dal[6:12, 0:n],
                                        in_=self.oun[96:97, :, 0:n])

                def t_rcp(self):
                    # reciprocal_approx_* requires fp32 in/out, so stage the
                    # bf16 denominators through fp32 (folding in the padding
                    # correction) and downcast the result for the 2x muls
                    n = self.n
                    dfl = nrmpool.tile([12, NFull], F32, tag="dfl",
                                       name=f"dfl{self.w}")
                    nc.vector.tensor_scalar_add(
                        dfl[:, 0:n], self.dal[:, 0:n],
                        float(NFull - self.n))
                    rcpf = nrmpool.tile([12, NFull], F32, tag="rcpf",
                                        name=f"rcpf{self.w}")
                    nc.vector.reciprocal_approx_fast(rcpf[:, 0:n],
                                                     dfl[:, 0:n])
                    rcp = nrmpool.tile([12, NFull], BF16, tag="rcp",
                                       name=f"rcp{self.w}")
                    self.rcp = rcp
                    nc.vector.tensor_copy(rcp[:, 0:n], rcpf[:, 0:n])

                def _bca_tile(self):
                    if not hasattr(self, "bca"):
                        # bca[64a+b, p, :] = 1/den of head 2p+a, so the mul
                        # input bases match oun's (same-start-partition rule)
                        self.bca = nbpool.tile([128, 6, NFull], BF16,
                                               tag="bca",
                                               name=f"bca{self.w}")

                def t_bcast(self):
                    n = self.n
                    dsc = dpool.tile([12, NFull], BF16, tag="dsc",
                                     name=f"dsc{self.w}")
                    nc.gpsimd.dma_start(out=dsc[:, 0:n], in_=self.rcp[:, 0:n])
                    self._bca_tile()
                    for a in range(2):
                        nc.gpsimd.dma_start(
                            out=self.bca[64 * a:64 * a + 32, :, 0:n],
                            in_=dsc[None, 6 * a:6 * a + 6, 0:n]
                            .to_broadcast((32, 6, n)))

                def tail_half(self, half):
                    # half-batched dal->rcp->broadcast chain for pairs
                    # 3*half..3*half+2, used for the last window so its tail
                    # pipelines into the pair loop instead of serializing
                    # after it (5 DMA triggers per half)
                    n = self.n
                    p0 = 3 * half
                    dal_h = nrmpool.tile([6, NFull], BF16, tag="dalh",
                                         name=f"dalh{self.w}_{half}")
                    dq = nc.scalar if self.in_last else nc.gpsimd
                    dq.dma_start(out=dal_h[0:3, 0:n],
                                 in_=self.oun[32:33, p0:p0 + 3, 0:n])
                    dq.dma_start(out=dal_h[3:6, 0:n],
                                 in_=self.oun[96:97, p0:p0 + 3, 0:n])
                    dfl_h = nrmpool.tile([6, NFull], F32, tag="dflh",
                                         name=f"dflh{self.w}_{half}")
                    nc.vector.tensor_scalar_add(
                        dfl_h[:, 0:n], dal_h[:, 0:n],
                        float(NFull - self.n))
                    rcpf_h = nrmpool.tile([6, NFull], F32, tag="rcpfh",
                                          name=f"rcpfh{self.w}_{half}")
                    nc.vector.reciprocal_approx_fast(rcpf_h[:, 0:n],
                                                     dfl_h[:, 0:n])
                    rcp_h = nrmpool.tile([6, NFull], BF16, tag="rcph",
                                         name=f"rcph{self.w}_{half}")
                    nc.vector.tensor_copy(rcp_h[:, 0:n], rcpf_h[:, 0:n])
                    dsc_h = dpool.tile([6, NFull], BF16, tag="dsch",
                                       name=f"dsch{self.w}_{half}")
                    nc.gpsimd.dma_start(out=dsc_h[:, 0:n], in_=rcp_h[:, 0:n])
                    self._bca_tile()
                    for a in range(2):
                        nc.gpsimd.dma_start(
                            out=self.bca[64 * a:64 * a + 32, p0:p0 + 3, 0:n],
                            in_=dsc_h[None, 3 * a:3 * a + 3, 0:n]
                            .to_broadcast((32, 3, n)))

                def t_mul(self, h):
                    n = self.n
                    ti, to = h // 4, 32 * (h % 4)
                    p, a = h // 2, h % 2
                    if not hasattr(self, "ot"):
                        self.ot = ozpool.tile([128, 3, NFull], BF16,
                                              tag="ot", name=f"ot{self.w}")
                    if self.in_last:
                        eng = nc.vector if h % 2 else nc.gpsimd
                    else:
                        eng = nc.vector if h in mul_dve else nc.gpsimd
                    eng.tensor_mul(
                        self.ot[to:to + 32, ti, 0:n],
                        self.oun[64 * a:64 * a + 32, p, 0:n],
                        self.bca[64 * a:64 * a + 32, p, 0:n])

                def t_proj(self, i):
                    n = self.n
                    if not hasattr(self, "zt"):
                        self.zt = ozpool.tile([128, 3, NFull], BF16,
                                              tag="zt", name=f"zt{self.w}")
                    pmm = ps_mm.tile([128, 512], F32, tag="mm")
                    for kk in range(3):
                        nc.tensor.matmul(pmm[:, 0:n],
                                         twp[:, kk, 128 * i:128 * i + 128],
                                         self.ot[:, kk, 0:n],
                                         start=(kk == 0), stop=(kk == 2))
                    with tc.high_priority(offset=10**6):
                        if self.in_last:
                            nc.scalar.add(self.zt[:, i, 0:n], pmm[:, 0:n],
                                          tpb[:, i:i + 1])
                        else:
                            nc.vector.tensor_scalar_add(self.zt[:, i, 0:n],
                                                        pmm[:, 0:n],
                                                        tpb[:, i:i + 1])

                def t_store(self):
                    nc.sync.dma_start(out=self.zout[self.si],
                                      in_=self.zt[:, :, 0:self.n])

                def tail_chunks(self):
                    out = [self.t_dal, self.t_rcp, self.t_bcast]
                    for h in range(NH):
                        out.append(lambda h=h: self.t_mul(h))
                    for i in range(3):
                        out.append(lambda i=i: self.t_proj(i))
                    out.append(self.t_store)
                    return out

            wins = [Window(w) for w in range(NW)]

            # ---- prologue: x + qkv for the first window pair; first
            # weights on the sync queue, the rest on the gpsimd queue so
            # transfers overlap ----
            # spread the prologue transfers across DMA queues so the
            # first QKV matmuls are ready ~2us in instead of ~7us
            nc.scalar.dma_start(out=twq[:], in_=wq[:])
            nc.gpsimd.dma_start(out=twk[:], in_=wk[:])
            for w in (4, 5):
                wins[w].load_x()
            for t, src in ((twv, wv), (twp, wp), (tpb, pb)):
                nc.gpsimd.dma_start(out=t[:], in_=src[:])
            for w in (4, 5):
                for c in wins[w].qkv_chunks():
                    c()

            NPAIR = NH // 2

            def window_steps(win, last):
                # one window's QK/exp/PV pipeline as a list of steps:
                # quads (4-strip QK + exps, two j at a time) interleaved
                # with the PVs of completed pairs
                def quad2(Q, j0):
                    win.qk_quad(Q, j0)
                    win.qk_quad(Q, j0 + 1)

                def pv_step(p, half0):
                    win.pv_pair(p)
                    if last and half0:
                        win.tail_half(0)
                        for hh in range(6):
                            win.t_mul(hh)

                steps = [
                    lambda: quad2(0, 0),
                    lambda: quad2(0, 2),
                    lambda: quad2(1, 0),
                    lambda: pv_step(0, False),
                    lambda: quad2(1, 2),
                    lambda: pv_step(1, False),
                    lambda: quad2(2, 0),
                    lambda: pv_step(2, False),
                    lambda: quad2(2, 2),
                    lambda: pv_step(3, last),
                    lambda: pv_step(4, False),
                    lambda: pv_step(5, False),
                ]
                return steps

            # Window groups run concurrently (their pair pipelines are
            # interleaved step by step), so one window's exp latency is
            # hidden by another's matmuls. Fillers (next group's x-load +
            # QKV, previous group's normalize/project tails) are spread
            # across the slots. The small windows (edges + corner) go
            # FIRST as one 3-wide group: their thin pipelines overlap each
            # other, and the last group is two full windows whose tails
            # interleave inline at the end.
            groups = [(4, 5), (6, 0), (1, 2), (3,)]
            for gi, grp in enumerate(groups):
                last_grp = gi == len(groups) - 1
                for w in grp:
                    wins[w].in_last = last_grp
                step_lists = [window_steps(wins[w], last_grp) for w in grp]
                merged = []
                for i in range(max(len(s) for s in step_lists)):
                    for s in step_lists:
                        if i < len(s):
                            merged.append(s[i])
                filler = []
                if gi + 1 < len(groups):
                    for nw in groups[gi + 1]:
                        filler.append(wins[nw].load_x)
                        filler.extend(wins[nw].qkv_chunks())
                if gi > 0:
                    tails = [wins[pw].tail_chunks() for pw in groups[gi - 1]]
                    for i in range(max(len(t) for t in tails)):
                        for t in tails:
                            if i < len(t):
                                filler.append(t[i])
                nslots = len(merged)
                per_slot = [[] for _ in range(nslots)]
                for idx, c in enumerate(filler):
                    per_slot[min(nslots - 1,
                                 idx * nslots // max(1, len(filler)))].append(c)
                for i, step in enumerate(merged):
                    step()
                    for c in per_slot[i]:
                        c()
                if last_grp:
                    for w in grp:
                        wins[w].tail_half(1)
                    for w in grp:
                        for hh in range(6, NH):
                            wins[w].t_mul(hh)
                    for w in grp:
                        for i in range(3):
                            wins[w].t_proj(i)
                        wins[w].t_store()

    nc.compile()
    return nc


WS = 10
NH = 12
C = 384
B, T, H, W = 2, 4, 44, 44
HG = WG = 5


def window_partition(x):
    """x: (B*T, H*W, C) -> windows (B, 25, 400, C) padded, plus metadata."""
    ax = x.reshape(B, T, H, W, C)
    pad = WS * HG
    axp = np.zeros((B, T, pad, pad, C), dtype=x.dtype)
    axp[:, :, :H, :W, :] = ax
    axp = axp.reshape(B, T, HG, WS, WG, WS, C)
    axp = axp.transpose(0, 2, 4, 1, 3, 5, 6).reshape(B, HG * WG, T * WS * WS, C)
    return axp


def classify_windows():
    """Return (full_list, edge_list, corner_list) of (b, w[, n_valid])."""
    full, edge, corner = [], [], []
    for b in range(B):
        for i in range(HG):
            for j in range(WG):
                w = i * WG + j
                vi = min(WS, H - i * WS)
                vj = min(WS, W - j * WS)
                nv = T * vi * vj
                if vi == WS and vj == WS:
                    full.append((b, w))
                elif nv <= NCorner:
                    corner.append((b, w, nv))
                else:
                    edge.append((b, w, nv))
    return full, edge, corner


def window_token_index(w):
    """For window w, indices of its 400 token slots ordered by (t, wi, wj),
    and validity mask."""
    i, j = w // WG, w % WG
    idx = np.zeros((T, WS, WS), dtype=np.int64)
    valid = np.zeros((T, WS, WS), dtype=bool)
    for t in range(T):
        for a in range(WS):
            for bb in range(WS):
                hh, ww = i * WS + a, j * WS + bb
                ok = (hh < H) and (ww < W)
                valid[t, a, bb] = ok
                idx[t, a, bb] = (t * H + min(hh, H - 1)) * W + min(ww, W - 1)
    return idx.reshape(-1), valid.reshape(-1)


def compact_window_tokens(xw, w):
    """xw: (400, C) padded window tokens (zeros at invalid). Returns
    (n_valid tokens compacted, order) where order lists the valid slot ids."""
    _, valid = window_token_index(w)
    order = np.nonzero(valid)[0]
    return xw[order], order


def _pack_tokens(xw_bw, nslot):
    """(400, C) padded window -> [128, 3, nslot] bf16 compacted tile."""
    return None  # placeholder, not used


def shard_inputs(x, qkv_w, proj_w, proj_b):
    """Build per-core in_maps. Returns (in_maps, meta) where meta is used by
    unshard."""
    x = np.asarray(x, dtype=np.float32)
    xw = window_partition(x)           # (B, 25, 400, C)
    full, edge, corner = classify_windows()
    assert len(full) == 32 and len(edge) == 16 and len(corner) == 2

    # per-core assignment: 4 full, 2 edge, and a corner on cores 0-1
    full_assign = [full[4 * c:4 * c + 4] for c in range(8)]
    edge_assign = [[] for _ in range(8)]
    for k, s in enumerate(edge):
        edge_assign[k % 8].append(s)
    corner_assign = [[] for _ in range(8)]
    for k, s in enumerate(corner):
        corner_assign[k].append(s)
    meta = {"full": full_assign, "edge": edge_assign,
            "corner": corner_assign, "orders": {}}

    wqT = qkv_w[0:C, :].T.astype(np.float32)      # (C, C): [c, qf]
    wkT = qkv_w[C:2 * C, :].T.astype(np.float32)
    wvT = qkv_w[2 * C:3 * C, :].T.astype(np.float32)
    wpT = proj_w.T.astype(np.float32)

    def wtile(wt):  # (C=384 rows c, C cols f) -> [128, 3, 384]
        return np.ascontiguousarray(
            wt.reshape(3, 128, C).transpose(1, 0, 2)).astype(ml_dtypes.bfloat16)

    def xtile(b, w, nv, nslot):
        toks, order = compact_window_tokens(xw[b, w], w)
        meta["orders"][(b, w)] = order
        xt = np.zeros((C, nslot), dtype=np.float32)
        xt[:, 0:nv] = toks.T
        return xt.reshape(3, 128, nslot).transpose(1, 0, 2).astype(
            ml_dtypes.bfloat16)

    in_maps = []
    for c in range(8):
        xfc = np.zeros((NF, 128, 3, NFull), dtype=ml_dtypes.bfloat16)
        for s, (b, w) in enumerate(full_assign[c]):
            xt = xw[b, w].T                      # (C, 400)
            xfc[s] = xt.reshape(3, 128, NFull).transpose(1, 0, 2).astype(
                ml_dtypes.bfloat16)
        xsc = np.zeros((NS, 128, 3, NSmall), dtype=ml_dtypes.bfloat16)
        for s, (b, w, nv) in enumerate(edge_assign[c]):
            xsc[s] = xtile(b, w, nv, NSmall)
        xcc = np.zeros((1, 128, 3, NCorner), dtype=ml_dtypes.bfloat16)
        for s, (b, w, nv) in enumerate(corner_assign[c]):
            xcc[s] = xtile(b, w, nv, NCorner)
        in_maps.append({
            "xf": xfc, "xs": xsc, "xc": xcc,
            "wq": wtile(wqT), "wk": wtile(wkT), "wv": wtile(wvT),
            "wp": wtile(wpT),
            "pb": np.ascontiguousarray(proj_b.astype(np.float32).reshape(3, 128).T),
        })
    return in_maps, meta


def unshard_outputs(results, meta):
    """results: list of 8 dicts with zf/zs/zc. Return (B*T, H*W, C)."""
    zwin = np.zeros((B, HG * WG, T * WS * WS, C), dtype=np.float32)
    for c in range(8):
        zfc = np.asarray(results[c]["zf"], dtype=np.float32)
        zsc = np.asarray(results[c]["zs"], dtype=np.float32)
        zcc = np.asarray(results[c]["zc"], dtype=np.float32)
        for s, (b, w) in enumerate(meta["full"][c]):
            zt = zfc[s].transpose(1, 0, 2).reshape(C, NFull)   # (C, 400)
            zwin[b, w] = zt.T
        for s, (b, w, nv) in enumerate(meta["edge"][c]):
            zt = zsc[s].transpose(1, 0, 2).reshape(C, NSmall)
            order = meta["orders"][(b, w)]
            zwin[b, w][order] = zt.T[0:nv]
        for s, (b, w, nv) in enumerate(meta["corner"][c]):
            zt = zcc[s].transpose(1, 0, 2).reshape(C, NCorner)
            order = meta["orders"][(b, w)]
            zwin[b, w][order] = zt.T[0:nv]
    # reverse window partition
    z = zwin.reshape(B, HG, WG, T, WS, WS, C)
    z = z.transpose(0, 3, 1, 4, 2, 5, 6).reshape(B, T, HG * WS, WG * WS, C)
    z = z[:, :, :H, :W, :]
    return z.reshape(B * T, H * W, C)


_CACHE = {}


def _get_nc():
    if "nc" not in _CACHE:
        _CACHE["nc"] = build_kernel()
    return _CACHE["nc"]


def kernel(x, qkv_w, proj_w, proj_b, t=4, H=44, W=44, **_unused):
    from concourse.bass_utils import run_bass_kernel_spmd

    x = np.asarray(x, dtype=np.float32)
    qkv_w = np.asarray(qkv_w, dtype=np.float32)
    proj_w = np.asarray(proj_w, dtype=np.float32)
    proj_b = np.asarray(proj_b, dtype=np.float32)
    in_maps, meta = shard_inputs(x, qkv_w, proj_w, proj_b)
    nc = _get_nc()
    res = run_bass_kernel_spmd(nc, in_maps, list(range(8)))
    return unshard_outputs(res.results, meta)


# revision 23
# speedup vs baseline: 1.2394x; 1.2394x over previous
"""Trainium2 Bass kernel for windowed (block-diagonal) multi-head attention.

Problem nn_Attention_17059610099953:
  x: (8, 1936, 384) tokens of a (B=2, t=4, H=44, W=44) volume; 10x10 spatial
  windows (padded to 50x50 -> 5x5 grid), each window = t*10*10 = 400 tokens of
  12-head attention (head_dim 32), followed by an output projection.

Sharding: 50 windows = 32 full (400 real tokens) + 16 edge (160) + 2 corner
(64). Each of the 8 NeuronCores processes 4 full windows + 2 edge windows
(compacted to 160 tokens) + 1 corner slot (64 tokens; zero for cores 2-7).
The softmax denominator is corrected by +(400 - slot_n) to match the
reference's 400-slot windows, whose zero padding tokens each contribute
exp(0)=1.

v3 pipeline (baseline v2 was 234us):
  - All matmul operands bf16; PSUM accumulation fp32.
  - QK^T runs pair-wise with PE row-strip tiling: heads (2p, 2p+1) target
    disjoint 32-row groups (tile_position), so their K=32 matmuls execute
    concurrently in the systolic array (~2x QK wall time).
  - exp is split between the Scalar engine (ACT exp LUT) and the Vector
    engine (Schraudolph int16 bitcast) per (head, j-group) half; the split
    is chosen to balance ACT vs DVE load within the 2e-2 error budget.
  - PV is column-tiled: heads 2p/2p+1 run in column strips (0,0)/(0,64);
    the 33rd stationary column is the all-ones softmax-denominator row.
  - PV output copies grab rows 0:97 in one instruction per pair (DVE cost
    scales with free size, not partitions), halving copy count.
  - All 12 per-head rescale muls run on GpSimd; normalization gathers and
    broadcasts ride the GpSimd DMA queue.
  - The PE instruction stream interleaves next-window QKV projections and
    prev-window tails between QK/PV groups to avoid >3.4us idle gaps that
    would re-throttle the HAM clock gate to 1.2 GHz.
"""
import os
import sys

for _p in ("/opt/trn_rl_repo",):
    if os.path.isdir(_p) and _p not in sys.path:
        sys.path.append(_p)

import numpy as np
import ml_dtypes

import concourse.bass as bass
import concourse.bacc as bacc
import concourse.mybir as mybir
import concourse.tile as tile

F32 = mybir.dt.float32
F32R = mybir.dt.float32r
BF16 = mybir.dt.bfloat16
I16 = mybir.dt.int16
AF = mybir.ActivationFunctionType
ALU = mybir.AluOpType

C = 384
NH = 12
HD = 32
SCALE = HD ** -0.5
NF = 4      # full windows per core (n=400)
NS = 2      # edge windows per core (n=160)
NFull = 400
NSmall = 160
NCorner = 64
VW = 33     # V columns per head (32 dims + ones column for the denominator)

# Schraudolph exp constants, bf16 flavor:
# exp(x) ~= bitcast_bf16(int16(x*EXPA16 + EXPB16))
EXPA16 = 184.6649652337873     # 2^7 / ln 2
EXPB16 = 127.0 * 128.0 - 5.585


def ceil_div(a, b):
    return (a + b - 1) // b


# exp units whose HIGH pair (tile B) is computed on the Vector engine
# (Schraudolph) while the low pair stays on Scalar. Keys are (quad, j).
# Splitting a unit across engines frees both score tiles simultaneously,
# which keeps the 4-strip QK quads concurrent; more units on DVE unloads
# ACT but raises the output error.
DVE_FULL = frozenset({(0, 2), (1, 3), (2, 1)})
DVE_SMALL = frozenset({(1, 1), (2, 0)})
DVE_CORNER = frozenset()
MUL_DVE = frozenset({1, 3, 5, 7, 9, 11})


def build_kernel(dve_full=DVE_FULL, dve_small=DVE_SMALL,
                 dve_corner=DVE_CORNER, mul_dve=MUL_DVE, n_warm=0):
    nc = bacc.Bacc("TRN2", target_bir_lowering=False, debug=False, num_devices=8)

    xf = nc.declare_dram_parameter("xf", [NF, 128, 3, NFull], BF16, isOutput=False)
    xs = nc.declare_dram_parameter("xs", [NS, 128, 3, NSmall], BF16, isOutput=False)
    xc = nc.declare_dram_parameter("xc", [1, 128, 3, NCorner], BF16, isOutput=False)
    wq = nc.declare_dram_parameter("wq", [128, 3, C], BF16, isOutput=False)
    wk = nc.declare_dram_parameter("wk", [128, 3, C], BF16, isOutput=False)
    wv = nc.declare_dram_parameter("wv", [128, 3, C], BF16, isOutput=False)
    wp = nc.declare_dram_parameter("wp", [128, 3, C], BF16, isOutput=False)
    pb = nc.declare_dram_parameter("pb", [128, 3], F32, isOutput=False)
    zf = nc.declare_dram_parameter("zf", [NF, 128, 3, NFull], BF16, isOutput=True)
    zs = nc.declare_dram_parameter("zs", [NS, 128, 3, NSmall], BF16, isOutput=True)
    zc = nc.declare_dram_parameter("zc", [1, 128, 3, NCorner], BF16, isOutput=True)

    slots = [(s, NFull, xf, zf, s, dve_full) for s in range(NF)] + \
            [(NF + s, NSmall, xs, zs, s, dve_small) for s in range(NS)] + \
            [(NF + NS, NCorner, xc, zc, 0, dve_corner)]
    NW = len(slots)

    with tile.TileContext(nc) as tc:
        with tc.tile_pool(name="weights", bufs=1) as wpool, \
             tc.tile_pool(name="xio", bufs=5) as xpool, \
             tc.tile_pool(name="qk", bufs=3) as qkpool, \
             tc.tile_pool(name="vaug", bufs=5) as vpool, \
             tc.tile_pool(name="es", bufs=8) as espool, \
             tc.tile_pool(name="oun", bufs=4) as ounpool, \
             tc.tile_pool(name="oz", bufs=4) as ozpool, \
             tc.tile_pool(name="nrm", bufs=3) as nrmpool, \
             tc.tile_pool(name="nrmbig", bufs=4) as nbpool, \
             tc.tile_pool(name="dscratch", bufs=6, space="DRAM") as dpool, \
             tc.tile_pool(name="ps_s", bufs=2, space="PSUM") as ps_s, \
             tc.tile_pool(name="ps_pv", bufs=2, space="PSUM") as ps_pv, \
             tc.tile_pool(name="ps_mm", bufs=2, space="PSUM") as ps_mm:

            twq = wpool.tile([128, 3, C], BF16, tag="wq")
            twk = wpool.tile([128, 3, C], BF16, tag="wk")
            twv = wpool.tile([128, 3, C], BF16, tag="wv")
            twp = wpool.tile([128, 3, C], BF16, tag="wp")
            tpb = wpool.tile([128, 3], F32, tag="pb")

            class Window:
                def __init__(self, w):
                    self.w = w
                    (self.slot, self.n, self.xin, self.zout, self.si,
                     self.dve_set) = slots[w]
                    self.n_mt = ceil_div(self.n, 128)
                    self.m_sizes = [min(128, self.n - 128 * j)
                                    for j in range(self.n_mt)]
                    self.full = self.n == NFull
                    self.in_last = False
                    self.es = {}
                    self.pss = {}

                # ---- stage 1: x load + QKV projections (run during w-1) ----
                def load_x(self):
                    self.xt = xpool.tile([128, 3, NFull], BF16, tag="xt",
                                         name=f"xt{self.w}")
                    nc.sync.dma_start(out=self.xt[:, :, 0:self.n],
                                      in_=self.xin[self.si])

                def qkv_chunks(self):
                    n = self.n
                    out = []

                    def qk_proj(dst_key, i, self=self):
                        if dst_key not in ("qt", "kt"):
                            raise ValueError
                        if not hasattr(self, dst_key):
                            setattr(self, dst_key,
                                    qkpool.tile([128, 3, NFull], BF16,
                                                tag=dst_key,
                                                name=f"{dst_key}{self.w}"))
                        dst = getattr(self, dst_key)
                        w_t = twq if dst_key == "qt" else twk
                        pmm = ps_mm.tile([128, 512], F32, tag="mm")
                        for kk in range(3):
                            nc.tensor.matmul(pmm[:, 0:n],
                                             w_t[:, kk, 128 * i:128 * i + 128],
                                             self.xt[:, kk, 0:n],
                                             start=(kk == 0), stop=(kk == 2))
                        with tc.high_priority(offset=10**6):
                            nc.vector.tensor_copy(dst[:, i, 0:n],
                                                  pmm[:, 0:n])

                    def v_tile(j, self=self):
                        if not hasattr(self, "vg"):
                            self.vg = vpool.tile([128, 4, NH * VW], BF16,
                                                 tag="vg", name=f"vg{self.w}")
                            vs = self.vg.rearrange("p j (h c) -> p j h c", h=NH)
                            nc.vector.memset(
                                vs[:, 0:self.n_mt, :, 32:33], 1.0)
                        mj = self.m_sizes[j]
                        pmm = ps_mm.tile([128, 512], F32, tag="mm")
                        for kk in range(3):
                            nc.tensor.matmul(pmm[0:mj, 0:C],
                                             self.xt[:, kk, 128 * j:128 * j + mj],
                                             twv[:, kk, :],
                                             start=(kk == 0), stop=(kk == 2))
                        vslice = self.vg[0:mj, j, :].rearrange(
                            "p (h c) -> p h c", h=NH)
                        with tc.high_priority(offset=10**6):
                            nc.vector.tensor_copy(
                                vslice[:, :, 0:32],
                                pmm[0:mj, 0:C].rearrange("p (h c) -> p h c",
                                                         h=NH))

                    for dst in ("qt", "kt"):
                        for i in range(3):
                            out.append(lambda d=dst, i=i: qk_proj(d, i))
                    for j in range(self.n_mt):
                        out.append(lambda j=j: v_tile(j))
                    return out

                # ---- stage 2: quad QK^T (4 row strips) + exp ----
                # Heads 4Q..4Q+3 run concurrently in the four 32-row strips
                # of the PE array (same kt/qt chunk ti=Q), writing 4
                # distinct PSUM banks: head pair (4Q,4Q+1) -> tile A slots
                # 0/1, (4Q+2,4Q+3) -> tile B. This uses the full array for
                # the K=32 score matmuls (4x fewer array-cycles than
                # serial heads).
                def qk_quad(self, Q, j):
                    n, n_mt = self.n, self.n_mt
                    if j >= n_mt:
                        return
                    mj = self.m_sizes[j]
                    p_lo, p_hi = 2 * Q, 2 * Q + 1
                    for p in (p_lo, p_hi):
                        if p not in self.es:
                            self.es[p] = espool.tile(
                                [128, 2, 4, NFull], BF16, tag="es",
                                name=f"es{self.w}_{p}")
                    tA = ps_s.tile([128, 2, 512], F32, tag="s",
                                   name=f"pssA{self.w}_{Q}_{j}")
                    tB = ps_s.tile([128, 2, 512], F32, tag="s",
                                   name=f"pssB{self.w}_{Q}_{j}")
                    for hi in range(4):
                        h = 4 * Q + hi
                        to = 32 * hi
                        dst = (tA, tB)[hi // 2][0:mj, hi % 2, 0:n]
                        nc.tensor.matmul(
                            dst,
                            self.kt[to:to + 32, Q, 128 * j:128 * j + mj],
                            self.qt[to:to + 32, Q, 0:n],
                            start=True, stop=True, tile_position=(to, 0))
                    for hi_pair, (p, t) in enumerate(((p_lo, tA),
                                                      (p_hi, tB))):
                        dst = self.es[p][:, 0:2, j, 0:n]
                        src = t[:, 0:2, 0:n]
                        with tc.high_priority(offset=10**6):
                            if (Q, j) in self.dve_set:
                                nc.vector.tensor_scalar(
                                    dst.bitcast(I16), src,
                                    SCALE * EXPA16, EXPB16, ALU.mult, ALU.add)
                            else:
                                nc.scalar.activation(dst, src, AF.Exp,
                                                     scale=SCALE)

                # ---- stage 3: PV for a head pair, column-tiled ----
                def pv_pair(self, p):
                    n, n_mt = self.n, self.n_mt
                    h0, h1 = 2 * p, 2 * p + 1
                    if not hasattr(self, "oun"):
                        self.oun = ounpool.tile([128, 6, NFull], BF16,
                                                tag="oun", name=f"oun{self.w}")
                    esp = self.es.pop(p)
                    ppv = ps_pv.tile([128, 512], F32, tag="pv")
                    for j in range(n_mt):
                        mj = self.m_sizes[j]
                        nc.tensor.matmul(
                            ppv[0:33, 0:n],
                            self.vg[0:mj, j, VW * h0:VW * h0 + VW],
                            esp[0:mj, 0, j, 0:n],
                            start=(j == 0), stop=(j == n_mt - 1),
                            tile_position=(0, 0), skip_group_check=True)
                        nc.tensor.matmul(
                            ppv[64:97, 0:n],
                            self.vg[0:mj, j, VW * h1:VW * h1 + VW],
                            esp[0:mj, 1, j, 0:n],
                            start=(j == 0), stop=(j == n_mt - 1),
                            tile_position=(0, 64), skip_group_check=True)
                    # one copy for both heads: DVE cost scales with the free
                    # dim, so grabbing rows 0:97 (33:64 are dead) is as cheap
                    # as one head's 0:33
                    with tc.high_priority(offset=10**6):
                        if self.in_last:
                            nc.scalar.copy(self.oun[0:97, p, 0:n],
                                           ppv[0:97, 0:n])
                        else:
                            nc.vector.tensor_copy(self.oun[0:97, p, 0:n],
                                                  ppv[0:97, 0:n])

                # ---- stage 4: normalize + project + store (run during w+1) --
                def t_dal(self):
                    n = self.n
                    self.dal = nrmpool.tile([12, NFull], BF16, tag="dal",
                                            name=f"dal{self.w}")
                    nc.gpsimd.dma_start(out=self.dal[0:6, 0:n],
                                        in_=self.oun[32:33, :, 0:n])
                    nc.gpsimd.dma_start(out=self.dal[6:12, 0:n],
                                        in_=self.oun[96:97, :, 0:n])

                def t_rcp(self):
                    # reciprocal_approx_* requires fp32 in/out, so stage the
                    # bf16 denominators through fp32 (folding in the padding
                    # correction) and downcast the result for the 2x muls
                    n = self.n
                    dfl = nrmpool.tile([12, NFull], F32, tag="dfl",
                                       name=f"dfl{self.w}")
                    nc.vector.tensor_scalar_add(
                        dfl[:, 0:n], self.dal[:, 0:n],
                        float(NFull - self.n))
                    rcpf = nrmpool.tile([12, NFull], F32, tag="rcpf",
                                        name=f"rcpf{self.w}")
                    nc.vector.reciprocal_approx_fast(rcpf[:, 0:n],
                                                     dfl[:, 0:n])
                    rcp = nrmpool.tile([12, NFull], BF16, tag="rcp",
                                       name=f"rcp{self.w}")
                    self.rcp = rcp
                    nc.vector.tensor_copy(rcp[:, 0:n], rcpf[:, 0:n])

                def _bca_tile(self):
                    if not hasattr(self, "bca"):
                        # bca[64a+b, p, :] = 1/den of head 2p+a, so the mul
                        # input bases match oun's (same-start-partition rule)
                        self.bca = nbpool.tile([128, 6, NFull], BF16,
                                               tag="bca",
                                               name=f"bca{self.w}")

                def t_bcast(self):
                    n = self.n
                    dsc = dpool.tile([12, NFull], BF16, tag="dsc",
                                     name=f"dsc{self.w}")
                    nc.gpsimd.dma_start(out=dsc[:, 0:n], in_=self.rcp[:, 0:n])
                    self._bca_tile()
                    for a in range(2):
                        nc.gpsimd.dma_start(
                            out=self.bca[64 * a:64 * a + 32, :, 0:n],
                            in_=dsc[None, 6 * a:6 * a + 6, 0:n]
                            .to_broadcast((32, 6, n)))

                def tail_half(self, half):
                    # half-batched dal->rcp->broadcast chain for pairs
                    # 3*half..3*half+2, used for the last window so its tail
                    # pipelines into the pair loop instead of serializing
                    # after it (5 DMA triggers per half)
                    n = self.n
                    p0 = 3 * half
                    dal_h = nrmpool.tile([6, NFull], BF16, tag="dalh",
                                         name=f"dalh{self.w}_{half}")
                    dq = nc.scalar if self.in_last else nc.gpsimd
                    dq.dma_start(out=dal_h[0:3, 0:n],
                                 in_=self.oun[32:33, p0:p0 + 3, 0:n])
                    dq.dma_start(out=dal_h[3:6, 0:n],
                                 in_=self.oun[96:97, p0:p0 + 3, 0:n])
                    dfl_h = nrmpool.tile([6, NFull], F32, tag="dflh",
                                         name=f"dflh{self.w}_{half}")
                    nc.vector.tensor_scalar_add(
                        dfl_h[:, 0:n], dal_h[:, 0:n],
                        float(NFull - self.n))
                    rcpf_h = nrmpool.tile([6, NFull], F32, tag="rcpfh",
                                          name=f"rcpfh{self.w}_{half}")
                    nc.vector.reciprocal_approx_fast(rcpf_h[:, 0:n],
                                                     dfl_h[:, 0:n])
                    rcp_h = nrmpool.tile([6, NFull], BF16, tag="rcph",
                                         name=f"rcph{self.w}_{half}")
                    nc.vector.tensor_copy(rcp_h[:, 0:n], rcpf_h[:, 0:n])
                    dsc_h = dpool.tile([6, NFull], BF16, tag="dsch",
                                       name=f"dsch{self.w}_{half}")
                    nc.gpsimd.dma_start(out=dsc_h[:, 0:n], in_=rcp_h[:, 0:n])
                    self._bca_tile()
                    for a in range(2):
                        nc.gpsimd.dma_start(
                            out=self.bca[64 * a:64 * a + 32, p0:p0 + 3, 0:n],
                            in_=dsc_h[None, 3 * a:3 * a + 3, 0:n]
                            .to_broadcast((32, 3, n)))

                def t_mul(self, h):
                    n = self.n
                    ti, to = h // 4, 32 * (h % 4)
                    p, a = h // 2, h % 2
                    if not hasattr(self, "ot"):
                        self.ot = ozpool.tile([128, 3, NFull], BF16,
                                              tag="ot", name=f"ot{self.w}")
                    if self.in_last:
                        eng = nc.vector if h % 2 else nc.gpsimd
                    else:
                        eng = nc.vector if h in mul_dve else nc.gpsimd
                    eng.tensor_mul(
                        self.ot[to:to + 32, ti, 0:n],
                        self.oun[64 * a:64 * a + 32, p, 0:n],
                        self.bca[64 * a:64 * a + 32, p, 0:n])

                def t_proj(self, i):
                    n = self.n
                    if not hasattr(self, "zt"):
                        self.zt = ozpool.tile([128, 3, NFull], BF16,
                                              tag="zt", name=f"zt{self.w}")
                    pmm = ps_mm.tile([128, 512], F32, tag="mm")
                    for kk in range(3):
                        nc.tensor.matmul(pmm[:, 0:n],
                                         twp[:, kk, 128 * i:128 * i + 128],
                                         self.ot[:, kk, 0:n],
                                         start=(kk == 0), stop=(kk == 2))
                    with tc.high_priority(offset=10**6):
                        if self.in_last:
                            nc.scalar.add(self.zt[:, i, 0:n], pmm[:, 0:n],
                                          tpb[:, i:i + 1])
                        else:
                            nc.vector.tensor_scalar_add(self.zt[:, i, 0:n],
                                                        pmm[:, 0:n],
                                                        tpb[:, i:i + 1])

                def t_store(self):
                    nc.sync.dma_start(out=self.zout[self.si],
                                      in_=self.zt[:, :, 0:self.n])

                def tail_chunks(self):
                    out = [self.t_dal, self.t_rcp, self.t_bcast]
                    for h in range(NH):
                        out.append(lambda h=h: self.t_mul(h))
                    for i in range(3):
                        out.append(lambda i=i: self.t_proj(i))
                    out.append(self.t_store)
                    return out

            wins = [Window(w) for w in range(NW)]

            # ---- prologue: x + qkv for the first window pair; first
            # weights on the sync queue, the rest on the gpsimd queue so
            # transfers overlap ----
            # spread the prologue transfers across DMA queues so the
            # first QKV matmuls are ready ~2us in instead of ~7us
            nc.scalar.dma_start(out=twq[:], in_=wq[:])
            nc.gpsimd.dma_start(out=twk[:], in_=wk[:])
            for w in (4, 5):
                wins[w].load_x()
            for t, src in ((twv, wv), (twp, wp), (tpb, pb)):
                nc.gpsimd.dma_start(out=t[:], in_=src[:])
            for w in (4, 5):
                for c in wins[w].qkv_chunks():
                    c()

            NPAIR = NH // 2

            def window_steps(win, last):
                # one window's QK/exp/PV pipeline as a list of steps:
                # quads (4-strip QK + exps, two j at a time) interleaved
                # with the PVs of completed pairs
                def quad2(Q, j0):
                    win.qk_quad(Q, j0)
                    win.qk_quad(Q, j0 + 1)

                def pv_step(p, half0):
                    win.pv_pair(p)
                    if last and half0:
                        win.tail_half(0)
                        for hh in range(6):
                            win.t_mul(hh)

                steps = [
                    lambda: quad2(0, 0),
                    lambda: quad2(0, 2),
                    lambda: quad2(1, 0),
                    lambda: pv_step(0, False),
                    lambda: quad2(1, 2),
                    lambda: pv_step(1, False),
                    lambda: quad2(2, 0),
                    lambda: pv_step(2, False),
                    lambda: quad2(2, 2),
                    lambda: pv_step(3, last),
                    lambda: pv_step(4, False),
                    lambda: pv_step(5, False),
                ]
                return steps

            # Window groups run concurrently (their pair pipelines are
            # interleaved step by step), so one window's exp latency is
            # hidden by another's matmuls. Fillers (next group's x-load +
            # QKV, previous group's normalize/project tails) are spread
            # across the slots. The small windows (edges + corner) go
            # FIRST as one 3-wide group: their thin pipelines overlap each
            # other, and the last group is two full windows whose tails
            # interleave inline at the end.
            groups = [(4, 5), (6, 0), (1, 2), (3,)]
            for gi, grp in enumerate(groups):
                last_grp = gi == len(groups) - 1
                for w in grp:
                    wins[w].in_last = False
                step_lists = [window_steps(wins[w], last_grp) for w in grp]
                merged = []
                for i in range(max(len(s) for s in step_lists)):
                    for s in step_lists:
                        if i < len(s):
                            merged.append(s[i])
                filler = []
                if gi + 1 < len(groups):
                    for nw in groups[gi + 1]:
                        filler.append(wins[nw].load_x)
                        filler.extend(wins[nw].qkv_chunks())
                if gi > 0:
                    tails = [wins[pw].tail_chunks() for pw in groups[gi - 1]]
                    for i in range(max(len(t) for t in tails)):
                        for t in tails:
                            if i < len(t):
                                filler.append(t[i])
                nslots = len(merged)
                per_slot = [[] for _ in range(nslots)]
                for idx, c in enumerate(filler):
                    per_slot[min(nslots - 1,
                                 idx * nslots // max(1, len(filler)))].append(c)
                for i, step in enumerate(merged):
                    step()
                    for c in per_slot[i]:
                        c()
                if last_grp:
                    for w in grp:
                        wins[w].tail_half(1)
                    for w in grp:
                        for hh in range(6, NH):
                            wins[w].t_mul(hh)
                    for w in grp:
                        for i in range(3):
                            wins[w].t_proj(i)
                        wins[w].t_store()

    nc.compile()
    return nc


WS = 10
NH = 12
C = 384
B, T, H, W = 2, 4, 44, 44
HG = WG = 5


def window_partition(x):
    """x: (B*T, H*W, C) -> windows (B, 25, 400, C) padded, plus metadata."""
    ax = x.reshape(B, T, H, W, C)
    pad = WS * HG
    axp = np.zeros((B, T, pad, pad, C), dtype=x.dtype)
    axp[:, :, :H, :W, :] = ax
    axp = axp.reshape(B, T, HG, WS, WG, WS, C)
    axp = axp.transpose(0, 2, 4, 1, 3, 5, 6).reshape(B, HG * WG, T * WS * WS, C)
    return axp


def classify_windows():
    """Return (full_list, edge_list, corner_list) of (b, w[, n_valid])."""
    full, edge, corner = [], [], []
    for b in range(B):
        for i in range(HG):
            for j in range(WG):
                w = i * WG + j
                vi = min(WS, H - i * WS)
                vj = min(WS, W - j * WS)
                nv = T * vi * vj
                if vi == WS and vj == WS:
                    full.append((b, w))
                elif nv <= NCorner:
                    corner.append((b, w, nv))
                else:
                    edge.append((b, w, nv))
    return full, edge, corner


def window_token_index(w):
    """For window w, indices of its 400 token slots ordered by (t, wi, wj),
    and validity mask."""
    i, j = w // WG, w % WG
    idx = np.zeros((T, WS, WS), dtype=np.int64)
    valid = np.zeros((T, WS, WS), dtype=bool)
    for t in range(T):
        for a in range(WS):
            for bb in range(WS):
                hh, ww = i * WS + a, j * WS + bb
                ok = (hh < H) and (ww < W)
                valid[t, a, bb] = ok
                idx[t, a, bb] = (t * H + min(hh, H - 1)) * W + min(ww, W - 1)
    return idx.reshape(-1), valid.reshape(-1)


def compact_window_tokens(xw, w):
    """xw: (400, C) padded window tokens (zeros at invalid). Returns
    (n_valid tokens compacted, order) where order lists the valid slot ids."""
    _, valid = window_token_index(w)
    order = np.nonzero(valid)[0]
    return xw[order], order


def _pack_tokens(xw_bw, nslot):
    """(400, C) padded window -> [128, 3, nslot] bf16 compacted tile."""
    return None  # placeholder, not used


def shard_inputs(x, qkv_w, proj_w, proj_b):
    """Build per-core in_maps. Returns (in_maps, meta) where meta is used by
    unshard."""
    x = np.asarray(x, dtype=np.float32)
    xw = window_partition(x)           # (B, 25, 400, C)
    full, edge, corner = classify_windows()
    assert len(full) == 32 and len(edge) == 16 and len(corner) == 2

    # per-core assignment: 4 full, 2 edge, and a corner on cores 0-1
    full_assign = [full[4 * c:4 * c + 4] for c in range(8)]
    edge_assign = [[] for _ in range(8)]
    for k, s in enumerate(edge):
        edge_assign[k % 8].append(s)
    corner_assign = [[] for _ in range(8)]
    for k, s in enumerate(corner):
        corner_assign[k].append(s)
    meta = {"full": full_assign, "edge": edge_assign,
            "corner": corner_assign, "orders": {}}

    wqT = qkv_w[0:C, :].T.astype(np.float32)      # (C, C): [c, qf]
    wkT = qkv_w[C:2 * C, :].T.astype(np.float32)
    wvT = qkv_w[2 * C:3 * C, :].T.astype(np.float32)
    wpT = proj_w.T.astype(np.float32)

    def wtile(wt):  # (C=384 rows c, C cols f) -> [128, 3, 384]
        return np.ascontiguousarray(
            wt.reshape(3, 128, C).transpose(1, 0, 2)).astype(ml_dtypes.bfloat16)

    def xtile(b, w, nv, nslot):
        toks, order = compact_window_tokens(xw[b, w], w)
        meta["orders"][(b, w)] = order
        xt = np.zeros((C, nslot), dtype=np.float32)
        xt[:, 0:nv] = toks.T
        return xt.reshape(3, 128, nslot).transpose(1, 0, 2).astype(
            ml_dtypes.bfloat16)

    in_maps = []
    for c in range(8):
        xfc = np.zeros((NF, 128, 3, NFull), dtype=ml_dtypes.bfloat16)
        for s, (b, w) in enumerate(full_assign[c]):
            xt = xw[b, w].T                      # (C, 400)
            xfc[s] = xt.reshape(3, 128, NFull).transpose(1, 0, 2).astype(
                ml_dtypes.bfloat16)
        xsc = np.zeros((NS, 128, 3, NSmall), dtype=ml_dtypes.bfloat16)
        for s, (b, w, nv) in enumerate(edge_assign[c]):
            xsc[s] = xtile(b, w, nv, NSmall)
        xcc = np.zeros((1, 128, 3, NCorner), dtype=ml_dtypes.bfloat16)
        for s, (b, w, nv) in enumerate(corner_assign[c]):
            xcc[s] = xtile(b, w, nv, NCorner)
        in_maps.append({
            "xf": xfc, "xs": xsc, "xc": xcc,
            "wq": wtile(wqT), "wk": wtile(wkT), "wv": wtile(wvT),
            "wp": wtile(wpT),
            "pb": np.ascontiguousarray(proj_b.astype(np.float32).reshape(3, 128).T),
        })
    return in_maps, meta


def unshard_outputs(results, meta):
    """results: list of 8 dicts with zf/zs/zc. Return (B*T, H*W, C)."""
    zwin = np.zeros((B, HG * WG, T * WS * WS, C), dtype=np.float32)
    for c in range(8):
        zfc = np.asarray(results[c]["zf"], dtype=np.float32)
        zsc = np.asarray(results[c]["zs"], dtype=np.float32)
        zcc = np.asarray(results[c]["zc"], dtype=np.float32)
        for s, (b, w) in enumerate(meta["full"][c]):
            zt = zfc[s].transpose(1, 0, 2).reshape(C, NFull)   # (C, 400)
            zwin[b, w] = zt.T
        for s, (b, w, nv) in enumerate(meta["edge"][c]):
            zt = zsc[s].transpose(1, 0, 2).reshape(C, NSmall)
            order = meta["orders"][(b, w)]
            zwin[b, w][order] = zt.T[0:nv]
        for s, (b, w, nv) in enumerate(meta["corner"][c]):
            zt = zcc[s].transpose(1, 0, 2).reshape(C, NCorner)
            order = meta["orders"][(b, w)]
            zwin[b, w][order] = zt.T[0:nv]
    # reverse window partition
    z = zwin.reshape(B, HG, WG, T, WS, WS, C)
    z = z.transpose(0, 3, 1, 4, 2, 5, 6).reshape(B, T, HG * WS, WG * WS, C)
    z = z[:, :, :H, :W, :]
    return z.reshape(B * T, H * W, C)


_CACHE = {}


def _get_nc():
    if "nc" not in _CACHE:
        _CACHE["nc"] = build_kernel()
    return _CACHE["nc"]


def kernel(x, qkv_w, proj_w, proj_b, t=4, H=44, W=44, **_unused):
    from concourse.bass_utils import run_bass_kernel_spmd

    x = np.asarray(x, dtype=np.float32)
    qkv_w = np.asarray(qkv_w, dtype=np.float32)
    proj_w = np.asarray(proj_w, dtype=np.float32)
    proj_b = np.asarray(proj_b, dtype=np.float32)
    in_maps, meta = shard_inputs(x, qkv_w, proj_w, proj_b)
    nc = _get_nc()
    res = run_bass_kernel_spmd(nc, in_maps, list(range(8)))
    return unshard_outputs(res.results, meta)
